# revision 17
# baseline (speedup 1.0000x reference)
"""GATv2 GNN (3 layers + per-graph max readout + MLP classifier) on 8 trn2 NeuronCores.

Sharding: nodes partitioned contiguously across 8 cores (6250 each). Edges are
bucketed by destination-node partition and sorted by dst; per dst-tile (128
nodes) the incoming edges form a contiguous run, padded to 128-edge chunks on a
schedule shared by all cores (SPMD: one program).

v3: all large host-built constant tensors (edge one-hot matrices PT/PTT, graph
masks, m01, broadcast biases) are constructed ON DEVICE from compact inputs
(per-edge dst-column indices, per-node graph ids, [1,*] rows) to cut per-core
input staging from 66MB to ~3.4MB.

Per layer, per core:
  node phase : h^T tiles via PE transpose -> hs/hd = h @ Wsrc/Wdst + b (PE, bf16)
  AllGather  : hs shards -> full hs table [50000,256] bf16 in local DRAM
  edge phase : per 128-edge chunk, indirect-DMA gather hs[src] and hd[dst_local];
               ef=lrelu(hs+hd) (DVE+ACT Prelu); e = <ef,attn> (DVE mult+reduce);
               ex=exp(e) (ACT); rhs=[ex*hs | ex]; segment-sum via PE matmul with
               a device-built one-hot edge->dst matrix PT into PSUM [128 dst, 264]
  update     : rst = num/denom ; h = relu(rst + h)
Readout: per-graph max over local nodes (PE transpose + DVE reduce, graphs are
contiguous since graph_ids is sorted), AllReduce(max), replicated classifier.
"""
import hashlib
import os
import numpy as np
import ml_dtypes

import concourse.bass as bass
import concourse.bacc as bacc
import concourse.tile as tile
import concourse.mybir as mybir
from concourse.masks import make_identity
from concourse.bass_utils import run_bass_kernel_spmd

F32 = mybir.dt.float32
BF16 = mybir.dt.bfloat16
I32 = mybir.dt.int32
BF = ml_dtypes.bfloat16

NCORES = 8
N = 50000
E = 800000
G = 64
IN_DIM = 128
HID = 256
HEADS = 8
DH = 32
OUT = 10
NEG = 0.2
NLOC = N // NCORES          # 6250
TILE = 128
NT = (NLOC + TILE - 1) // TILE   # 49
NLOC_PAD = NT * TILE             # 6272
GRP = 16                         # chunks per group (DVE/ACT amortization)
EPS = 1e-30


# ----------------------------------------------------------------------------- host prep

def _preprocess(x, src, dst, graph_ids):
    """Build per-core compact shard data + the shared chunk schedule."""
    src = np.asarray(src, np.int64)
    dst = np.asarray(dst, np.int64)
    gid = np.asarray(graph_ids, np.int64)

    core_of = dst // NLOC
    # per-core sorted edge lists, split by src parity (fixed 64/64 lane quota per chunk)
    per_core = []
    Ecnt = np.zeros((NCORES, NT), np.int64)
    Ocnt = np.zeros((NCORES, NT), np.int64)
    for c in range(NCORES):
        m = core_of == c
        s_c = src[m]
        d_c = dst[m] - c * NLOC
        o = np.argsort(d_c, kind="stable")
        s_c, d_c = s_c[o], d_c[o]
        per_core.append((s_c, d_c))
        tiles = d_c // TILE
        odd = (s_c & 1).astype(bool)
        Ecnt[c] = np.bincount(tiles[~odd], minlength=NT)
        Ocnt[c] = np.bincount(tiles[odd], minlength=NT)

    C = np.maximum(
        np.maximum.reduce([np.ceil(Ecnt[c] / 64).astype(np.int64) for c in range(NCORES)]),
        np.maximum.reduce([np.ceil(Ocnt[c] / 64).astype(np.int64) for c in range(NCORES)]),
    )
    assert (C > 0).all(), "some dst tile has zero edges on all cores"
    CC = int(C.sum())
    NGRP = (CC + GRP - 1) // GRP
    CCg = NGRP * GRP
    C = C.copy()
    C[NT - 1] += CCg - CC           # pad chunks appended to the last tile
    CC = CCg
    NGRP = CC // GRP
    chunk_tile = np.repeat(np.arange(NT), C)        # [CC]
    tile_chunk_start = np.concatenate([[0], np.cumsum(C)])[:-1]

    srcidx = np.zeros((NCORES, 128, CC), np.int32)
    srcidx[:, 64:, :] = 1                            # odd-half pads default to src=1
    # colv[lane, chunk]: dst-in-tile column for the edge in that slot, 999 for pads
    colv = np.full((NCORES, 128, CC), 999.0, np.float32)
    for c in range(NCORES):
        s_c, d_c = per_core[c]
        tiles = d_c // TILE
        odd = (s_c & 1).astype(bool)
        # rank within (tile, parity)
        slot = np.empty(len(s_c), np.int64)
        for par in (0, 1):
            mm = odd == bool(par)
            tt = tiles[mm]
            cnt = np.bincount(tt, minlength=NT)
            start_e = np.concatenate([[0], np.cumsum(cnt)])[:-1]
            rank = np.arange(mm.sum()) - start_e[tt]
            lane = par * 64 + (rank % 64)
            chk = tile_chunk_start[tt] + rank // 64
            slot[mm] = chk * 128 + lane
        si = np.zeros(CC * 128, np.int32)
        si[:] = np.tile(np.concatenate([np.zeros(64, np.int32), np.ones(64, np.int32)]), CC)
        si[slot] = s_c
        srcidx[c] = si.reshape(CC, 128).T
        cv = np.full(CC * 128, 999.0, np.float32)
        cv[slot] = (d_c - tiles * TILE).astype(np.float32)
        colv[c] = cv.reshape(CC, 128).T
    # dma_gather (pair-trick) index packing, compact: only the [16, NGRP*128]
    # master block is shipped; the device replicates it to all 128 partitions.
    NI = GRP * 128
    srcidx16 = np.zeros((NCORES, 16, NGRP * (NI // 16)), np.int16)
    for c in range(NCORES):
        si_flat = srcidx[c].T.reshape(-1)        # slot-major [CC*128]
        half = (si_flat >> 1).astype(np.int16).reshape(NGRP, NI)
        blk = half.reshape(NGRP, NI // 16, 16).transpose(0, 2, 1)   # [NGRP, 16, NI//16]
        srcidx16[c] = blk.transpose(1, 0, 2).reshape(16, NGRP * (NI // 16))

    # per-core transposed x, zero-padded to NLOC_PAD
    x = np.asarray(x, np.float32)
    xT = np.zeros((NCORES, IN_DIM, NLOC_PAD), BF)
    for c in range(NCORES):
        xT[c, :, :NLOC] = x[c * NLOC:(c + 1) * NLOC].T.astype(BF)

    meta = dict(CC=CC, NGRP=NGRP, C=C, chunk_tile=chunk_tile)
    shards = dict(srcidx16=srcidx16, colv=colv, xT=xT)
    return meta, shards


def _pack_weights(Win, b_in, Wsrc, bsrc, Wdst, bdst, attn, Wc1, bc1, Wc2, bc2, Wc3, bc3):
    w = {}
    w["Win_bf"] = np.asarray(Win, np.float32).astype(BF)                       # [128,256]
    Wsrc = np.asarray(Wsrc, np.float32)
    Wdst = np.asarray(Wdst, np.float32)
    # [l, p, kb*256+j] = W[l, kb*128+p, j]
    w["Wsrc_pk"] = Wsrc.reshape(3, 2, 128, 256).transpose(0, 2, 1, 3).reshape(3, 128, 512).astype(BF)
    w["Wdst_pk"] = Wdst.reshape(3, 2, 128, 256).transpose(0, 2, 1, 3).reshape(3, 128, 512).astype(BF)
    w["attn_row"] = np.asarray(attn, np.float32).reshape(1, 3 * HID).copy()     # [1,768] f32
    w["b_in_row"] = np.asarray(b_in, np.float32).reshape(1, HID).copy()         # [1,256]
    w["bsrc_row"] = np.asarray(bsrc, np.float32).reshape(1, 3 * HID).copy()
    w["bdst_row"] = np.asarray(bdst, np.float32).reshape(1, 3 * HID).copy()
    Wc1 = np.asarray(Wc1, np.float32)
    # [p, (kb*2+mb)*128+j] = Wc1[kb*128+p, mb*128+j]
    w["Wc1_pk"] = Wc1.reshape(2, 128, 2, 128).transpose(1, 0, 2, 3).reshape(128, 512).astype(BF)
    Wc2 = np.asarray(Wc2, np.float32)
    w["Wc2_pk"] = Wc2.reshape(2, 128, 128).transpose(1, 0, 2).reshape(128, 256).astype(BF)
    w["Wc3_bf"] = np.asarray(Wc3, np.float32).astype(BF)                        # [128,10]
    w["bc1_col"] = np.asarray(bc1, np.float32).reshape(2, 128).T.copy()         # [128,2]
    w["bc2_col"] = np.asarray(bc2, np.float32).reshape(128, 1).copy()           # [128,1]
    w["bc3_bc"] = np.broadcast_to(np.asarray(bc3, np.float32)[None, :], (G, OUT)).copy()
    return w


# ----------------------------------------------------------------------------- builder

def build_program_v2(meta, debug_taps=False, sim_mode=False, phases=('proj','node','ag','edge','readout'), repeat=1):
    """SPMD-safe build. Large one-hot / mask / broadcast tensors are built on
    device from compact inputs:

      PT/PTT  : per 128-edge chunk one-hot [edge, dst] / [dst, edge], built by
                DVE is_equal(iota_row, colv_col) + PE transpose, staged once to
                local DRAM, re-read per layer by the edge phase.
      masks   : per-slot node->graph one-hot rows from gid_row via is_equal,
                partition-broadcast with a K=1 PE matmul against a ones column.
      m01     : pure-tile/graph indicator, broadcast the same way from m01_row.
      biases  : [1,256] f32 rows broadcast to [128,256] via K=1 f32 matmul.
    """
    CC, NGRP = meta["CC"], meta["NGRP"]
    C, chunk_tile = meta["C"], meta["chunk_tile"]
    slots = meta["slots"]          # list of (local_tile, graph) global slot schedule
    NSLOT = len(slots)

    KO = set((os.environ.get("GAT_KO") or "").split(",")) - {""}
    nc = bacc.Bacc("TRN2", target_bir_lowering=False, debug=False,
                   num_devices=(1 if sim_mode else NCORES))

    # ---- inputs
    ap = {}
    def din(name, shape, dt):
        ap[name] = nc.dram_tensor(name, shape, dt, kind="ExternalInput").ap()
    din("xT", [IN_DIM, NLOC_PAD], BF16)
    din("srcidx16", [16, NGRP * GRP * 8], mybir.dt.int16)
    din("colv", [128, CC], F32)
    din("gid_row", [1, NT * TILE], BF16)
    din("m01_row", [1, G * NT], BF16)
    din("Win_bf", [128, 256], BF16)
    din("Wsrc_pk", [3, 128, 512], BF16)
    din("Wdst_pk", [3, 128, 512], BF16)
    din("attn_row", [1, 3 * HID], F32)
    din("b_in_row", [1, HID], F32)
    din("bsrc_row", [1, 3 * HID], F32)
    din("bdst_row", [1, 3 * HID], F32)
    din("Wc1_pk", [128, 512], BF16)
    din("Wc2_pk", [128, 256], BF16)
    din("Wc3_bf", [128, OUT], BF16)
    din("bc1_col", [128, 2], F32)
    din("bc2_col", [128, 1], F32)
    din("bc3_bc", [G, OUT], F32)

    out_ap = nc.dram_tensor("out", [G, OUT], F32, kind="ExternalOutput").ap()
    taps = {}
    if debug_taps:
        for nm in ("h0", "h1", "h2", "h3"):
            taps[nm] = nc.dram_tensor(nm, [NLOC_PAD, HID], F32, kind="ExternalOutput").ap()
        taps["gmax"] = nc.dram_tensor("tap_gmax", [128, 128], F32, kind="ExternalOutput").ap()

    # ---- internal DRAM
    hs_bounce = [nc.dram_tensor(f"hs_bounce{l}", [NLOC, HID], BF16, kind="Internal").ap() for l in range(3)]
    hs_full = [nc.dram_tensor(f"hs_full{l}", [N, HID], BF16, kind="Internal", addr_space="Shared").ap() for l in range(3)]

    srcidx_dram = nc.dram_tensor("srcidx_dram", [128, NGRP * GRP * 8], mybir.dt.int16, kind="Internal").ap()
    gm_in = nc.dram_tensor("gm_in", [128, 128], F32, kind="Internal").ap()
    gm_out = nc.dram_tensor("gm_out", [128, 128], F32, kind="Internal", addr_space="Shared").ap()

    with tile.TileContext(nc) as tc:
        with (
            tc.tile_pool(name="const", bufs=1) as cp,
            tc.tile_pool(name="hbuf", bufs=1) as hp,
            tc.tile_pool(name="node", bufs=2) as npl,
            tc.tile_pool(name="edge", bufs=2) as ep,
            tc.tile_pool(name="upd", bufs=2) as up,
            tc.tile_pool(name="ro", bufs=1) as rp,
            tc.tile_pool(name="psA", bufs=2, space="PSUM") as psA,   # transposes
            tc.tile_pool(name="psB", bufs=2, space="PSUM") as psB,   # node matmuls
            tc.tile_pool(name="psS", bufs=2, space="PSUM") as psS,   # segment accum
        ):
            # ------- constants to SBUF
            def load_const(name, shape, dt):
                t = cp.tile(shape, dt, name=f"c_{name}", tag=f"c_{name}")
                nc.sync.dma_start(t[:], ap[name][:])
                return t
            # srcidx16: replicate the [16, X] master block to 128 rows in DRAM
            nc.sync.dma_start(srcidx_dram[0:16, :], ap["srcidx16"][:])
            nc.sync.dma_start(srcidx_dram[16:32, :], srcidx_dram[0:16, :])
            nc.sync.dma_start(srcidx_dram[32:64, :], srcidx_dram[0:32, :])
            nc.sync.dma_start(srcidx_dram[64:128, :], srcidx_dram[0:64, :])
            colv_f = load_const("colv", [128, CC], F32)
            Win_sb = load_const("Win_bf", [128, 256], BF16)
            Wsrc_sb = [None] * 3
            Wdst_sb = [None] * 3
            for l in range(3):
                Wsrc_sb[l] = cp.tile([128, 512], BF16, tag=f"wsrc{l}", name=f"wsrc{l}")
                nc.sync.dma_start(Wsrc_sb[l][:], ap["Wsrc_pk"][l])
                Wdst_sb[l] = cp.tile([128, 512], BF16, tag=f"wdst{l}", name=f"wdst{l}")
                nc.sync.dma_start(Wdst_sb[l][:], ap["Wdst_pk"][l])
            Wc1_sb = load_const("Wc1_pk", [128, 512], BF16)
            Wc2_sb = load_const("Wc2_pk", [128, 256], BF16)
            Wc3_sb = load_const("Wc3_bf", [128, OUT], BF16)
            bc1_sb = load_const("bc1_col", [128, 2], F32)
            bc2_sb = load_const("bc2_col", [128, 1], F32)
            bc3_sb = load_const("bc3_bc", [G, OUT], F32)
            ident = cp.tile([128, 128], F32)
            make_identity(nc, ident)
            identb = cp.tile([128, 128], BF16, tag="identb", name="identb")
            nc.vector.tensor_copy(identb[:], ident[:])
            alpha_sb = cp.tile([128, 1], F32)
            nc.vector.memset(alpha_sb[:], NEG)
            ones1f = cp.tile([1, 128], F32, tag="ones1f", name="ones1f")
            nc.vector.memset(ones1f[:], 1.0)
            ones1b = cp.tile([1, 128], BF16, tag="ones1b", name="ones1b")
            nc.vector.memset(ones1b[:], 1.0)
            iota_i = cp.tile([128, 128], I32, tag="iota_i", name="iota_i")
            nc.gpsimd.iota(iota_i[:], [[1, 128]], channel_multiplier=0)
            iota_f = cp.tile([128, 128], BF16, tag="iota_f", name="iota_f")
            nc.vector.tensor_copy(iota_f[:], iota_i[:])

            def bcast_row(dram_name, col0, ncols, out_tile, ps_pool, tag, dt=F32):
                """out_tile[:, :ncols] = DRAM row broadcast to 128 partitions (K=1 matmul)."""
                done = 0
                while done < ncols:
                    w = min(256, ncols - done)
                    rowld = npl.tile([1, 256], dt, tag=f"rowld{1 if dt == F32 else 2}")
                    nc.sync.dma_start(rowld[0:1, :w],
                                      ap[dram_name][0:1, col0 + done:col0 + done + w])
                    ps = ps_pool.tile([128, 256], F32, tag=tag)
                    nc.tensor.matmul(out=ps[:, :w],
                                     lhsT=(ones1f if dt == F32 else ones1b)[:],
                                     rhs=rowld[0:1, :w],
                                     start=True, stop=True)
                    nc.vector.tensor_copy(out_tile[:, done:done + w], ps[:, :w])
                    done += w

            # broadcast biases / attn / m01 to [128, *] SBUF
            binc_sb = cp.tile([128, 256], F32, tag="binc", name="binc")
            bcast_row("b_in_row", 0, 256, binc_sb, psB, "mmA")
            bsrc_sb = [None] * 3
            bdst_sb = [None] * 3
            attn_sb = [None] * 3
            for l in range(3):
                bsrc_sb[l] = cp.tile([128, 256], F32, tag=f"bsrc{l}", name=f"bsrc{l}")
                bcast_row("bsrc_row", l * 256, 256, bsrc_sb[l], psB, "mmA")
                bdst_sb[l] = cp.tile([128, 256], F32, tag=f"bdst{l}", name=f"bdst{l}")
                bcast_row("bdst_row", l * 256, 256, bdst_sb[l], psB, "mmB")
                attn_f = npl.tile([128, 256], F32, tag="attnf")
                bcast_row("attn_row", l * 256, 256, attn_f, psB, "mmA")
                attn_sb[l] = cp.tile([128, 256], BF16, tag=f"attn{l}", name=f"attn{l}")
                nc.vector.tensor_copy(attn_sb[l][:], attn_f[:])
            m01_sb = cp.tile([128, G * NT], BF16, tag="m01", name="m01")
            bcast_row("m01_row", 0, G * NT, m01_sb, psB, "mmB", dt=BF16)

            h_sb = hp.tile([128, NT * 256], F32)
            hd_all = hp.tile([128, NT * 256], BF16)


            def _forward(_rep=0):
                # ------- phase 0: input projection
                for t in range(NT if 'proj' in phases else 0):
                    xt_t = npl.tile([128, 128], BF16, tag="xt_t")
                    nc.sync.dma_start(xt_t[:], ap["xT"][:, t * 128:(t + 1) * 128])
                    ps = psB.tile([128, 256], F32, tag="mmA")
                    nc.tensor.matmul(out=ps[:], lhsT=xt_t[:],
                                     rhs=Win_sb[:], start=True, stop=True)
                    nc.vector.tensor_tensor(out=h_sb[:, t * 256:(t + 1) * 256],
                                            in0=ps[:], in1=binc_sb[:], op=mybir.AluOpType.add)
                if debug_taps:
                    for t in range(NT):
                        nc.sync.dma_start(taps["h0"][t * 128:(t + 1) * 128, :], h_sb[:, t * 256:(t + 1) * 256])

                # ------- layers
                for l in range(3 if ('node' in phases) else 0):
                    # node phase
                    for t in range(NT):
                        hT = npl.tile([128, 256], BF16, tag="hT")
                        for kb in range(2):
                            tp = psA.tile([128, 128], F32, tag="tp")
                            nc.tensor.transpose(out=tp[:], in_=h_sb[:, t * 256 + kb * 128: t * 256 + (kb + 1) * 128],
                                                identity=ident[:])
                            nc.vector.tensor_copy(hT[:, kb * 128:(kb + 1) * 128], tp[:])
                        hs_ps = psB.tile([128, 256], F32, tag="mmA")
                        hd_ps = psB.tile([128, 256], F32, tag="mmB")
                        for kb in range(2):
                            nc.tensor.matmul(out=hs_ps[:], lhsT=hT[:, kb * 128:(kb + 1) * 128],
                                             rhs=Wsrc_sb[l][:, kb * 256:(kb + 1) * 256],
                                             start=(kb == 0), stop=(kb == 1))
                        for kb in range(2):
                            nc.tensor.matmul(out=hd_ps[:], lhsT=hT[:, kb * 128:(kb + 1) * 128],
                                             rhs=Wdst_sb[l][:, kb * 256:(kb + 1) * 256],
                                             start=(kb == 0), stop=(kb == 1))
                        hs_o = npl.tile([128, 256], BF16, tag="hs_o")
                        nc.vector.tensor_tensor(out=hs_o[:], in0=hs_ps[:], in1=bsrc_sb[l][:], op=mybir.AluOpType.add)
                        nc.vector.tensor_tensor(out=hd_all[:, t * 256:(t + 1) * 256],
                                                in0=hd_ps[:], in1=bdst_sb[l][:], op=mybir.AluOpType.add)
                        rows = min(TILE, NLOC - t * TILE)
                        nc.sync.dma_start(hs_bounce[l][t * TILE:t * TILE + rows, :], hs_o[:rows, :])

                    if 'ag' in phases:
                        if sim_mode:
                            nc.sync.dma_start(hs_full[l][0:NLOC, :], hs_bounce[l][:])
                        else:
                            nc.gpsimd.collective_compute(
                                "AllGather", mybir.AluOpType.bypass,
                                replica_groups=[list(range(NCORES))],
                                ins=[hs_bounce[l][:]], outs=[hs_full[l][:]],
                            )

                    # edge phase
                    open_ps = {}
                    done = np.zeros(NT, np.int64)
                    for g in range(NGRP if 'edge' in phases else 0):
                        pt = ep.tile([128, GRP * 128], BF16, tag="pt")
                        ptt = ep.tile([128, GRP * 128], BF16, tag="ptt")
                        for j in range(GRP):
                            ch = g * GRP + j
                            ptf = ep.tile([128, 128], F32, tag="ptf")
                            nc.vector.tensor_scalar(
                                out=ptf[:],
                                in0=iota_f[:],
                                scalar1=colv_f[:, ch:ch + 1],
                                scalar2=None,
                                op0=mybir.AluOpType.is_equal)
                            nc.vector.tensor_copy(pt[:, j * 128:(j + 1) * 128], ptf[:])
                            tp = psA.tile([128, 128], F32, tag="tp")
                            nc.tensor.transpose(out=tp[:], in_=ptf[:], identity=ident[:])
                            nc.vector.tensor_copy(ptt[:, j * 128:(j + 1) * 128], tp[:])
                        NCOL = GRP * 8
                        sg16 = ep.tile([128, NCOL], mybir.dt.int16, tag="sg16")
                        nc.sync.dma_start(sg16[:], srcidx_dram[:, g * NCOL:(g + 1) * NCOL])
                        hsg = ep.tile([128, GRP * 256], BF16, tag="hsg")
                        pair = ep.tile([128, GRP * 512], BF16, tag="pair")
                        if "gather" in KO:
                            nc.vector.memset(pair[:], 0.03)
                        if "gather" not in KO:
                            H = GRP // 2
                            for hb in range(2):
                                nc.gpsimd.dma_gather(
                                    out_ap=pair[:, hb * H * 512:(hb + 1) * H * 512]
                                        .rearrange("p (c d) -> p c d", d=512),
                                    in_ap=hs_full[l][:].rearrange("(a b) c -> a (b c)", b=2),
                                    idxs_ap=sg16[:, hb * (NCOL // 2):(hb + 1) * (NCOL // 2)],
                                    num_idxs=H * 128, num_idxs_reg=H * 128, elem_size=512)
                        # lanes [0,64) carry even-src edges (pair row 0), [64,128) odd (row 1)
                        pair_i = pair[:].bitcast(mybir.dt.int32)
                        hsg_i = hsg[:].bitcast(mybir.dt.int32)
                        pvi = pair_i.rearrange("p (c s) -> p c s", s=256)
                        nc.vector.tensor_copy(
                            hsg_i.rearrange("p (c d) -> p c d", d=128)[0:64],
                            pvi[0:64, :, 0:128])
                        nc.vector.tensor_copy(
                            hsg_i.rearrange("p (c d) -> p c d", d=128)[64:128],
                            pvi[64:128, :, 128:256])
                        eflr = ep.tile([128, GRP * 256], BF16, tag="pair")
                        for j in range(GRP if "ef" not in KO else 0):
                            ch = g * GRP + j
                            t = int(chunk_tile[ch])
                            efps = psB.tile([128, 256], F32, tag="mmA", name=f"ef_l{l}_r{_rep}_c{ch}")
                            nc.tensor.matmul(out=efps[:], lhsT=ptt[:, j * 128:(j + 1) * 128],
                                             rhs=hd_all[:, t * 256:(t + 1) * 256],
                                             start=True, stop=False)
                            nc.tensor.matmul(out=efps[:], lhsT=identb[:],
                                             rhs=hsg[:, j * 256:(j + 1) * 256],
                                             start=False, stop=True)
                            nc.scalar.activation(out=eflr[:, j * 256:(j + 1) * 256], in_=efps[:],
                                                 func=mybir.ActivationFunctionType.Prelu,
                                                 alpha=alpha_sb[:, 0:1])
                        rhs = ep.tile([128, GRP * 264], BF16, tag="rhs")
                        if "dve" not in KO:
                            attn3 = attn_sb[l][:].rearrange("p (o c) -> p o c", o=1).to_broadcast([128, GRP, 256])
                            nc.vector.tensor_tensor(
                                out=eflr[:].rearrange("p (j c) -> p j c", c=256),
                                in0=eflr[:].rearrange("p (j c) -> p j c", c=256),
                                in1=attn3, op=mybir.AluOpType.mult)
                            e32 = ep.tile([128, GRP * 8], F32, tag="e32")
                            nc.vector.tensor_reduce(out=e32[:], in_=eflr[:].rearrange("p (a d) -> p a d", d=32),
                                                    axis=mybir.AxisListType.X, op=mybir.AluOpType.add)
                            nc.scalar.activation(
                                out=rhs[:].rearrange("p (j c) -> p j c", c=264)[:, :, 256:264],
                                in_=e32[:].rearrange("p (j c) -> p j c", c=8),
                                func=mybir.ActivationFunctionType.Exp)
                            nc.vector.tensor_tensor(
                                out=rhs[:].rearrange("p (j c) -> p j c", c=264)[:, :, 0:256]
                                    .rearrange("p j (h d) -> p j h d", d=32),
                                in0=hsg[:].rearrange("p (j h d) -> p j h d", h=8, d=32),
                                in1=rhs[:].rearrange("p (j c) -> p j c", c=264)[:, :, 256:264]
                                    .rearrange("p j (h o) -> p j h o", o=1).to_broadcast([128, GRP, 8, 32]),
                                op=mybir.AluOpType.mult)
                        for j in range(GRP):
                            ch = g * GRP + j
                            t = int(chunk_tile[ch])
                            if t not in open_ps:
                                open_ps[t] = psS.tile([128, 264], F32, tag="seg", name=f"seg_l{l}_t{t}")
                            first = done[t] == 0
                            done[t] += 1
                            last = done[t] == C[t]
                            nc.tensor.matmul(out=open_ps[t][:],
                                             lhsT=pt[:, j * 128:(j + 1) * 128],
                                             rhs=rhs[:, j * 264:(j + 1) * 264],
                                             start=first, stop=last)
                            if last:
                                ps = open_ps.pop(t)
                                den = up.tile([128, 8], F32, tag="den")
                                nc.vector.tensor_scalar_add(out=den[:], in0=ps[:, 256:264], scalar1=EPS)
                                rec = up.tile([128, 8], F32, tag="rec")
                                nc.vector.reciprocal(rec[:], den[:])
                                updt = up.tile([128, 256], F32, tag="updt")
                                nc.vector.tensor_tensor(
                                    out=updt[:].rearrange("p (h d) -> p h d", d=32),
                                    in0=ps[:, 0:256].rearrange("p (h d) -> p h d", d=32),
                                    in1=rec[:].rearrange("p (h o) -> p h o", o=1).to_broadcast([128, 8, 32]),
                                    op=mybir.AluOpType.mult)
                                nc.vector.tensor_tensor(out=updt[:], in0=updt[:],
                                                        in1=h_sb[:, t * 256:(t + 1) * 256],
                                                        op=mybir.AluOpType.add)
                                nc.scalar.activation(out=h_sb[:, t * 256:(t + 1) * 256], in_=updt[:],
                                                     func=mybir.ActivationFunctionType.Relu)
                    if debug_taps:
                        for t in range(NT):
                            nc.sync.dma_start(taps[f"h{l + 1}"][t * 128:(t + 1) * 128, :], h_sb[:, t * 256:(t + 1) * 256])

                # ------- readout: per-graph max (feat-major), slots are global schedule
                do_ro = 'readout' in phases
                gmax = rp.tile([128, 128], F32, tag="gmax")
                nc.vector.memset(gmax[:], 0.0)
                # group slots by tile so we transpose each tile once per block
                from collections import defaultdict
                by_tile = defaultdict(list)
                for si, (t, g) in enumerate(slots):
                    by_tile[t].append((si, g))
                TM = rp.tile([128, 2 * NT], F32, tag="TM", name=f"TM_{_rep}")
                for t in (range(NT) if do_ro else []):
                    hTt = rp.tile([128, 256], F32, tag="hTt", name=f"hTt_{_rep}_{t}")
                    for kb in range(2):
                        tp = psA.tile([128, 128], F32, tag="tp")
                        nc.tensor.transpose(out=tp[:], in_=h_sb[:, t * 256 + kb * 128: t * 256 + (kb + 1) * 128],
                                            identity=ident[:])
                        nc.vector.tensor_copy(hTt[:, kb * 128:(kb + 1) * 128], tp[:])
                    # pure per-tile max (both blocks): TM[:, kb*NT + t]
                    nc.vector.tensor_reduce(
                        out=TM[:].rearrange("p (b q) -> p b q", q=NT)[:, :, t:t + 1],
                        in_=hTt[:].rearrange("p (b n) -> p b n", n=128),
                        axis=mybir.AxisListType.X, op=mybir.AluOpType.max)
                    if t not in by_tile:
                        continue
                    # device-built masks: mask row via is_equal on gid_row, then
                    # partition-broadcast via K=1 matmul
                    grow = rp.tile([1, 128], BF16, tag="grow", name=f"grow_{_rep}_{t}")
                    nc.sync.dma_start(grow[0:1, :], ap["gid_row"][0:1, t * 128:(t + 1) * 128])
                    for _k, (si, g) in enumerate(by_tile[t]):
                        mrow = rp.tile([1, 128], BF16, tag="mrow", name=f"mrow_{_rep}_{si}")
                        nc.vector.tensor_scalar(
                            out=mrow[:],
                            in0=grow[0:1, :],
                            scalar1=float(g), scalar2=None,
                            op0=mybir.AluOpType.is_equal)
                        mps = psA.tile([128, 128], F32, tag="tp")
                        nc.tensor.matmul(out=mps[:], lhsT=ones1b[:], rhs=mrow[:],
                                         start=True, stop=True)
                        mskd = rp.tile([128, 256], F32, tag="mskd")
                        nc.vector.tensor_tensor(
                            out=mskd[:].rearrange("p (b n) -> p b n", n=128),
                            in0=hTt[:].rearrange("p (b n) -> p b n", n=128),
                            in1=mps[:].rearrange("p (o n) -> p o n", o=1).to_broadcast([128, 2, 128]),
                            op=mybir.AluOpType.mult)
                        red = rp.tile([128, 2], F32, tag="red")
                        nc.vector.tensor_reduce(out=red[:], in_=mskd[:].rearrange("p (b n) -> p b n", n=128),
                                                axis=mybir.AxisListType.X, op=mybir.AluOpType.max)
                        gsl = gmax[:].rearrange("p (b q) -> p b q", q=64)[:, :, g:g + 1]
                        nc.vector.tensor_tensor(out=gsl, in0=gsl,
                                                in1=red[:].rearrange("p (b o) -> p b o", o=1),
                                                op=mybir.AluOpType.max)
                # combine pure tile maxima per graph
                for g in (range(G) if do_ro else []):
                    mg = rp.tile([128, 2 * NT], F32, tag="mg", name=f"mg_{_rep}_{g}")
                    nc.vector.tensor_tensor(
                        out=mg[:].rearrange("p (b q) -> p b q", q=NT),
                        in0=TM[:].rearrange("p (b q) -> p b q", q=NT),
                        in1=m01_sb[:, g * NT:(g + 1) * NT]
                            .rearrange("p (o q) -> p o q", o=1).to_broadcast([128, 2, NT]),
                        op=mybir.AluOpType.mult)
                    redg = rp.tile([128, 2], F32, tag="redg", name=f"redg_{_rep}_{g}")
                    nc.vector.tensor_reduce(out=redg[:], in_=mg[:].rearrange("p (b q) -> p b q", q=NT),
                                            axis=mybir.AxisListType.X, op=mybir.AluOpType.max)
                    gsl = gmax[:].rearrange("p (b q) -> p b q", q=64)[:, :, g:g + 1]
                    nc.vector.tensor_tensor(out=gsl, in0=gsl,
                                            in1=redg[:].rearrange("p (b o) -> p b o", o=1),
                                            op=mybir.AluOpType.max)
                nc.sync.dma_start(gm_in[:], gmax[:])
                if sim_mode:
                    nc.sync.dma_start(gm_out[:], gm_in[:])
                else:
                    nc.gpsimd.collective_compute(
                        "AllReduce", mybir.AluOpType.max,
                        replica_groups=[list(range(NCORES))],
                        ins=[gm_in[:]], outs=[gm_out[:]],
                    )
                gT = rp.tile([128, 128], F32, tag="gT")
                nc.sync.dma_start(gT[:], gm_out[:])
                if debug_taps:
                    nc.sync.dma_start(taps["gmax"][:], gT[:])
                gTb = rp.tile([128, 128], BF16, tag="gTb")
                nc.vector.tensor_copy(gTb[:], gT[:])

                # ------- classifier (replicated)
                z1 = rp.tile([128, 128], BF16, tag="z1")
                for mb in range(2):
                    ps = psB.tile([128, 64], F32, tag="mmA")
                    for kb in range(2):
                        nc.tensor.matmul(out=ps[:], lhsT=Wc1_sb[:, (kb * 2 + mb) * 128:(kb * 2 + mb + 1) * 128],
                                         rhs=gTb[:, kb * 64:(kb + 1) * 64], start=(kb == 0), stop=(kb == 1))
                    nc.scalar.activation(out=z1[:, mb * 64:(mb + 1) * 64], in_=ps[:],
                                         func=mybir.ActivationFunctionType.Relu,
                                         bias=bc1_sb[:, mb:mb + 1], scale=1.0)
                ps2 = psB.tile([128, 64], F32, tag="mmB")
                for kb in range(2):
                    nc.tensor.matmul(out=ps2[:], lhsT=Wc2_sb[:, kb * 128:(kb + 1) * 128],
                                     rhs=z1[:, kb * 64:(kb + 1) * 64], start=(kb == 0), stop=(kb == 1))
                z2 = rp.tile([128, 64], BF16, tag="z2")
                nc.scalar.activation(out=z2[:], in_=ps2[:], func=mybir.ActivationFunctionType.Relu,
                                     bias=bc2_sb[:, 0:1], scale=1.0)
                ps3 = psB.tile([64, OUT], F32, tag="mmA")
                nc.tensor.matmul(out=ps3[:], lhsT=z2[:, 0:64], rhs=Wc3_sb[:, 0:OUT], start=True, stop=True)
                out_sb = rp.tile([64, OUT], F32, tag="osb")
                nc.vector.tensor_tensor(out=out_sb[:], in0=ps3[:], in1=bc3_sb[:], op=mybir.AluOpType.add)
                nc.sync.dma_start(out_ap[:], out_sb[:])
            for _rep in range(repeat):
                _forward(_rep)


    nc.compile()
    return nc


# ----------------------------------------------------------------------------- slots + rows

def _build_slots_and_masks(graph_ids):
    """Global (local_tile, graph) slot schedule + compact per-core rows.

    Slots enumerate, over the GLOBAL node order, each (owning-core tile,
    graph) incidence for IMPURE tiles (not 128 rows of one graph). The
    instruction schedule is identical on every core; per-core gid_row data
    makes non-owning cores produce all-zero masks (gid_row pads = -1).
    Pure tiles are handled via per-tile maxima TM + the m01 indicator.
    """
    gid = np.asarray(graph_ids, np.int64)
    slots = []           # (local_tile, graph)
    owners = []          # owning core
    for c in range(NCORES):
        g_c = gid[c * NLOC:(c + 1) * NLOC]
        for t in range(NT):
            lo = t * TILE
            hi = min((t + 1) * TILE, NLOC)
            if lo >= hi:
                continue
            gt = g_c[lo:hi]
            for g in np.unique(gt):
                slots.append((t, int(g)))
                owners.append(c)
    order = sorted(range(len(slots)), key=lambda i: slots[i][0])
    slots = [slots[i] for i in order]
    owners = [owners[i] for i in order]
    # tile purity per (core, tile): full 128 real rows, single graph
    m01_row = np.zeros((NCORES, 1, G * NT), BF)
    impure = set()
    for c in range(NCORES):
        g_c = gid[c * NLOC:(c + 1) * NLOC]
        for t in range(NT):
            lo, hi = t * TILE, min((t + 1) * TILE, NLOC)
            gt = np.unique(g_c[lo:hi])
            if hi - lo == 128 and len(gt) == 1:
                m01_row[c, 0, int(gt[0]) * NT + t] = 1.0
            else:
                impure.add((c, t))
    # impure slots: global schedule of (owner, tile, graph) incidences.
    # A slot emits instructions on EVERY core; only the owner's gid_row
    # matches graph g in tile t, others contribute 0 (h>=0 post-relu).
    slots2 = []
    for (t2, g2), c2 in zip(slots, owners):
        if (c2, t2) in impure:
            slots2.append((t2, g2))
    # the same (t, g) may occur on several cores; instructions are data-driven
    # by gid_row so duplicates are redundant — dedup keeps the schedule short.
    seen = set()
    slots = []
    for s in slots2:
        if s not in seen:
            seen.add(s)
            slots.append(s)
    if not slots:
        slots = [(0, 0)]
    # per-core gid_row: graph id per node (tile-major layout), pads = -1.
    # For IMPURE-slot masking each core must only match nodes it owns.
    gid_row = np.full((NCORES, 1, NT * TILE), -1.0, np.float32)  # cast to bf16 below
    for c in range(NCORES):
        g_c = gid[c * NLOC:(c + 1) * NLOC].astype(np.float32)
        gid_row[c, 0, :NLOC] = g_c
        # nodes in PURE tiles are already covered by TM+m01; masking them out
        # of slot matching is unnecessary (slot max <= true max, AllReduce-max
        # fixes it) but keeps slot semantics exact for impure tiles only.
    return slots, gid_row.astype(BF), m01_row



# ----------------------------------------------------------------------------- entry

_CACHE = {}

def _prepare(inputs, debug_taps=False):
    """Preprocess + build (cached on graph structure). Returns (nc, meta, in_maps)."""
    x = np.asarray(inputs["x"], np.float32)
    src = np.asarray(inputs["src"], np.int32)
    dst = np.asarray(inputs["dst"], np.int32)
    graph_ids = np.asarray(inputs["graph_ids"], np.int32)

    key = (hashlib.sha1(np.ascontiguousarray(src).tobytes()
                        + np.ascontiguousarray(dst).tobytes()
                        + np.ascontiguousarray(graph_ids).tobytes()).hexdigest(),
           debug_taps)
    if key in _CACHE:
        nc, meta = _CACHE[key]
        # x-dependent shard (xT) must be rebuilt if x changed; cheap enough to redo
        meta["shards"]["xT"] = _x_shard(x)
    else:
        meta, shards = _preprocess(x, src, dst, graph_ids)
        slots, gid_row, m01_row = _build_slots_and_masks(graph_ids)
        meta["slots"] = slots
        meta["gid_row"] = gid_row
        meta["m01_row"] = m01_row
        meta["shards"] = shards
        nc = build_program_v2(meta, debug_taps=debug_taps)
        _CACHE[key] = (nc, meta)
    shards = meta["shards"]

    w = _pack_weights(inputs["Win"], inputs["b_in"], inputs["Wsrc"], inputs["bsrc"],
                      inputs["Wdst"], inputs["bdst"], inputs["attn"],
                      inputs["Wc1"], inputs["bc1"], inputs["Wc2"], inputs["bc2"],
                      inputs["Wc3"], inputs["bc3"])

    in_maps = []
    for c in range(NCORES):
        m = dict(
            xT=shards["xT"][c], srcidx16=shards["srcidx16"][c],
            colv=shards["colv"][c],
            gid_row=meta["gid_row"][c], m01_row=meta["m01_row"][c],
            Win_bf=w["Win_bf"], Wsrc_pk=w["Wsrc_pk"], Wdst_pk=w["Wdst_pk"],
            attn_row=w["attn_row"], b_in_row=w["b_in_row"], bsrc_row=w["bsrc_row"],
            bdst_row=w["bdst_row"], Wc1_pk=w["Wc1_pk"], Wc2_pk=w["Wc2_pk"],
            Wc3_bf=w["Wc3_bf"], bc1_col=w["bc1_col"], bc2_col=w["bc2_col"],
            bc3_bc=w["bc3_bc"],
        )
        in_maps.append(m)
    return nc, meta, in_maps


def _x_shard(x):
    xT = np.zeros((NCORES, IN_DIM, NLOC_PAD), BF)
    for c in range(NCORES):
        xT[c, :, :NLOC] = x[c * NLOC:(c + 1) * NLOC].T.astype(BF)
    return xT


def kernel(**inputs):
    nc, meta, in_maps = _prepare(inputs)
    res = run_bass_kernel_spmd(nc, in_maps, core_ids=list(range(NCORES)))
    return np.asarray(res.results[0]["out"], np.float32)
